# revision 5
# baseline (speedup 1.0000x reference)
"""Trainium2 Bass kernel: cross-attention block (1x1-conv projections + MHA).

Full computation (reference semantics, fp32 inputs):
    q = x @ Wq.T + bq;  k,v = context @ Wkv.T + bkv (split)
    per head: out_h = softmax(q_h @ k_h.T * scale) @ v_h
    out = concat_heads @ Wo.T + bo

Sharding: 8 cores = 4 batches x 2 head-groups (4 heads each).  Each core
computes its batch/head-group partial of the output projection; the host
sums the two head-group partials per batch (the "all-reduce") and adds bo.

Per-core kernel (n = m = 2048, d = 256, local inner e = 256):
  - transpose x, ctx on PE (via identity) to get d-on-partition layouts
  - qT[e,n], kT[e,m] via matmuls (bias via K=1 rank-1 matmul), v[m,e] natural
  - v' = [v_h | 1] per head (65 cols): attn@v matmul also yields the
    softmax denominator as row 64 for free
  - simT[j,i] = kT_h.T @ qT_h per head (K=64), exp on ACT (scale folded
    into Wq on host), av accumulation over j-tiles, normalize with
    reciprocal + gpsimd partition_broadcast + DVE multiply
  - out-projection partial from oT (e-on-partition) with per-head K=64 slabs
Matmuls run as float32r (TF32-like, 1 cy/row at N>=256 vs 4 for fp32).
"""

import sys

if "/opt/trn_rl_repo" not in sys.path:
    sys.path.insert(0, "/opt/trn_rl_repo")

from contextlib import ExitStack

import numpy as np

import concourse.bacc as bacc
import concourse.tile as tile
from concourse import mybir
from concourse.bass_utils import run_bass_kernel_spmd

f32 = mybir.dt.float32
f32r = mybir.dt.float32r

B = 4          # global batch
N = 2048       # query sequence
MSEQ = 2048    # context sequence
D = 256        # query/context feature dim
HEADS = 8      # global heads
EH = 4         # heads per core (head-group)
DH = 64        # head dim
E = EH * DH    # per-core inner dim (256)
OD = 256       # output dim
SCALE = DH ** -0.5
NCORES = 8

NT = N // 128      # 16 query 128-tiles
MT = MSEQ // 128   # 16 context 128-tiles
KD = D // 128      # 2 contraction tiles over d
NB = N // 512      # 4 query 512-blocks

_CACHE = {}


def _build():
    nc = bacc.Bacc()
    x = nc.declare_dram_parameter("x", [N, D], f32, isOutput=False)
    cx = nc.declare_dram_parameter("cx", [MSEQ, D], f32, isOutput=False)
    wq = nc.declare_dram_parameter("wq", [D, E], f32, isOutput=False)
    wk = nc.declare_dram_parameter("wk", [D, E], f32, isOutput=False)
    wv = nc.declare_dram_parameter("wv", [D, E], f32, isOutput=False)
    wo = nc.declare_dram_parameter("wo", [EH, DH, OD], f32, isOutput=False)
    bq = nc.declare_dram_parameter("bq", [1, E], f32, isOutput=False)
    bk = nc.declare_dram_parameter("bk", [1, E], f32, isOutput=False)
    bv = nc.declare_dram_parameter("bv", [1, E], f32, isOutput=False)
    cst = nc.declare_dram_parameter("cst", [128, 640], f32, isOutput=False)
    out = nc.declare_dram_parameter("out", [N, OD], f32, isOutput=True)

    with tile.TileContext(nc) as tc, ExitStack() as ctx:
        P = ctx.enter_context(tc.tile_pool(name="persist", bufs=1))

        cst_sb = P.tile([128, 640], f32r)
        nc.sync.dma_start(out=cst_sb, in_=cst[:, :].bitcast(f32r))
        ident = cst_sb[:, 0:128]
        ones = cst_sb[0:1, 128:640]

        wq_sb = P.tile([128, KD, E], f32r)
        wk_sb = P.tile([128, KD, E], f32r)
        wv_sb = P.tile([128, KD, E], f32r)
        wo_sb = P.tile([64, EH, OD], f32r)
        nc.sync.dma_start(out=wq_sb, in_=wq.rearrange("(k p) e -> p k e", p=128).bitcast(f32r))
        nc.sync.dma_start(out=wk_sb, in_=wk.rearrange("(k p) e -> p k e", p=128).bitcast(f32r))
        nc.sync.dma_start(out=wv_sb, in_=wv.rearrange("(k p) e -> p k e", p=128).bitcast(f32r))
        nc.sync.dma_start(out=wo_sb, in_=wo.rearrange("h p o -> p h o").bitcast(f32r))
        bq_sb = P.tile([1, E], f32r)
        bk_sb = P.tile([1, E], f32r)
        bv_sb = P.tile([1, E], f32r)
        nc.sync.dma_start(out=bq_sb, in_=bq[:, :].bitcast(f32r))
        nc.sync.dma_start(out=bk_sb, in_=bk[:, :].bitcast(f32r))
        nc.sync.dma_start(out=bv_sb, in_=bv[:, :].bitcast(f32r))

        xT = P.tile([128, KD, N], f32r)     # x.T  (d on partitions)
        cT = P.tile([128, KD, MSEQ], f32r)  # ctx.T
        qT = P.tile([128, KD, N], f32r)     # q.T  (e on partitions)
        kT = P.tile([128, KD, MSEQ], f32r)  # k.T
        vS = P.tile([128, MT, EH, DH + 1], f32r)  # v' with ones column per head
        oT = P.tile([64, EH, N], f32r)      # attention out, e on partitions 0-63

        # ---- phase A: transposes + projections -------------------------
        with tc.tile_pool(name="stage", bufs=1) as S, \
             tc.tile_pool(name="psA", bufs=2, space="PSUM") as PSA:
            xs = S.tile([128, NT, D], f32r)
            cs = S.tile([128, MT, D], f32r)
            nc.sync.dma_start(out=xs, in_=x.rearrange("(t p) d -> p t d", p=128).bitcast(f32r))
            nc.sync.dma_start(out=cs, in_=cx.rearrange("(t p) d -> p t d", p=128).bitcast(f32r))

            for src, dst, nt in ((xs, xT, NT), (cs, cT, MT)):
                for t in range(nt):
                    for k in range(KD):
                        pt = PSA.tile([128, 128], f32r, tag="pT")
                        nc.tensor.transpose(pt, src[:, t, k * 128:(k + 1) * 128], ident)
                        nc.vector.tensor_copy(dst[:, k, t * 128:(t + 1) * 128], pt)

            for w_sb, b_sb, src, dst, nblk in (
                (wq_sb, bq_sb, xT, qT, NB),
                (wk_sb, bk_sb, cT, kT, MSEQ // 512),
            ):
                for m in range(KD):
                    for t in range(nblk):
                        pq = PSA.tile([128, 512], f32, tag="pA")
                        nc.tensor.matmul(pq, b_sb[0:1, m * 128:(m + 1) * 128],
                                         ones[0:1, 0:512], start=True, stop=False)
                        for k in range(KD):
                            nc.tensor.matmul(pq, w_sb[:, k, m * 128:(m + 1) * 128],
                                             src[:, k, t * 512:(t + 1) * 512],
                                             start=False, stop=(k == KD - 1))
                        nc.vector.tensor_copy(dst[:, m, t * 512:(t + 1) * 512], pq)

            nc.vector.tensor_copy(
                vS[:, :, :, 64],
                cst_sb[:, 128:192].rearrange("p (a b) -> p a b", a=MT))
            for mt in range(MT):
                pv = PSA.tile([128, 512], f32, tag="pA")
                nc.tensor.matmul(pv[:, 0:E], ones[0:1, 0:128], bv_sb[0:1, :],
                                 start=True, stop=False)
                for k in range(KD):
                    nc.tensor.matmul(pv[:, 0:E], cT[:, k, mt * 128:(mt + 1) * 128],
                                     wv_sb[:, k, :], start=False, stop=(k == KD - 1))
                nc.vector.tensor_copy(
                    vS[:, mt, :, 0:DH],
                    pv[:, 0:E].rearrange("p (h c) -> p h c", h=EH))

        # ---- phase B: attention + out-projection -----------------------
        with tc.tile_pool(name="psS", bufs=2, space="PSUM") as PSS, \
             tc.tile_pool(name="psV", bufs=3, space="PSUM") as PSV, \
             tc.tile_pool(name="psO", bufs=1, space="PSUM") as PSO, \
             tc.tile_pool(name="expp", bufs=3) as EX, \
             tc.tile_pool(name="smallp", bufs=4) as SM, \
             tc.tile_pool(name="outs", bufs=3) as OS:
            for ii in range(NB):
                for h in range(EH):
                    hp, hh = divmod(h, 2)
                    po = hh * 64
                    av = PSV.tile([128, 512], f32, tag="av")
                    for jp in range(MT // 2):
                        sp = PSS.tile([128, 1024], f32, tag="sim")
                        for u in range(2):
                            jj = jp * 2 + u
                            nc.tensor.matmul(
                                sp[:, u * 512:(u + 1) * 512],
                                kT[po:po + 64, hp, jj * 128:(jj + 1) * 128],
                                qT[po:po + 64, hp, ii * 512:(ii + 1) * 512],
                                start=True, stop=True)
                        ex = EX.tile([128, 1024], f32r, tag="exp")
                        nc.scalar.activation(ex, sp, mybir.ActivationFunctionType.Exp)
                        for u in range(2):
                            jj = jp * 2 + u
                            nc.tensor.matmul(
                                av[0:DH + 1, :], vS[:, jj, h, :],
                                ex[:, u * 512:(u + 1) * 512],
                                start=(jj == 0), stop=(jj == MT - 1),
                                skip_group_check=True)
                    rc = SM.tile([65, 512], f32, tag="rc")
                    nc.vector.reciprocal(rc[64:65, :], av[DH:DH + 1, :])
                    r0 = SM.tile([1, 512], f32, tag="r0")
                    nc.sync.dma_start(out=r0, in_=rc[64:65, :])
                    bc = SM.tile([64, 512], f32, tag="bc")
                    nc.gpsimd.partition_broadcast(bc, r0)
                    nc.vector.tensor_mul(oT[:, h, ii * 512:(ii + 1) * 512],
                                         av[0:DH, :], bc)

                for nt in range(ii * 4, ii * 4 + 4):
                    pob = PSO.tile([128, 256], f32, tag="op")
                    for h in range(EH):
                        nc.tensor.matmul(pob, oT[:, h, nt * 128:(nt + 1) * 128],
                                         wo_sb[:, h, :], start=(h == 0),
                                         stop=(h == EH - 1))
                    ot = OS.tile([128, 256], f32, tag="ot")
                    nc.vector.tensor_copy(ot, pob)
                    nc.sync.dma_start(out=out[nt * 128:(nt + 1) * 128, :], in_=ot)

    nc.finalize()
    return nc


def _get_nc():
    if "nc" not in _CACHE:
        _CACHE["nc"] = _build()
    return _CACHE["nc"]


def _make_in_maps(x, context, Wq, bq, Wkv, bkv, Wo, bo):
    f = np.float32
    inner = HEADS * DH
    cst = np.zeros((128, 640), dtype=f)
    cst[:, 0:128] = np.eye(128, dtype=f)
    cst[:, 128:640] = 1.0
    in_maps = []
    for c in range(NCORES):
        b, g = divmod(c, 2)
        sl = slice(g * E, (g + 1) * E)
        slv = slice(inner + g * E, inner + (g + 1) * E)
        woT = np.ascontiguousarray(Wo[:, sl].T, dtype=f)          # [E, OD]
        in_maps.append({
            "x": np.ascontiguousarray(x[b], dtype=f),
            "cx": np.ascontiguousarray(context[b], dtype=f),
            "wq": np.ascontiguousarray((Wq[sl] * SCALE).T, dtype=f),
            "wk": np.ascontiguousarray(Wkv[sl].T, dtype=f),
            "wv": np.ascontiguousarray(Wkv[slv].T, dtype=f),
            "wo": np.ascontiguousarray(woT.reshape(EH, DH, OD), dtype=f),
            "bq": np.ascontiguousarray((bq[sl] * SCALE).reshape(1, E), dtype=f),
            "bk": np.ascontiguousarray(bkv[sl].reshape(1, E), dtype=f),
            "bv": np.ascontiguousarray(bkv[slv].reshape(1, E), dtype=f),
            "cst": cst,
        })
    return in_maps


def _run(in_maps, trace=False, tmpdir=None):
    nc = _get_nc()
    return run_bass_kernel_spmd(nc, in_maps, list(range(NCORES)),
                                trace=trace, tmpdir=tmpdir)


def kernel(x, context, Wq, bq, Wkv, bkv, Wo, bo):
    in_maps = _make_in_maps(x, context, Wq, bq, Wkv, bkv, Wo, bo)
    res = _run(in_maps)
    parts = [r["out"] for r in res.results]
    bo_f = np.asarray(bo, dtype=np.float32)
    full = np.stack([parts[2 * b] + parts[2 * b + 1] + bo_f for b in range(B)])
    return full.astype(np.float32)
